# revision 1
# baseline (speedup 1.0000x reference)
"""CTC loss kernel for Trainium2 (Bass/Tile), 8-core data-parallel over batch.

Algorithm (per core, 4 batches):
  - lse path: ACT exp+accum per [128t, V] logits tile -> ln -> accumulate
    sum_t log(sum_v exp(logits[b,t,v])) per batch (no max-sub; logits ~N(0,1)).
  - gather: PE transpose logits tiles to [v, t], matmul with host-built
    one-hot E[v, s] -> G[s, t] = logits[b, t, ext[b,s]]; ACT exp -> q
    (unnormalized probs); DMA to DRAM staging buffer [b, s', t] (s' = s+16,
    zero-padded) then group-DMA back in DP layout.
  - DP: probability-domain CTC forward, 1024 serial steps on DVE.
    State alpha[s] and z[s] = alpha[s]*m'[s] (m'[s] = allow_skip[s+2]) in one
    [128, 2W] tile: partition 32*qd + b holds S-chunk qd of batch b, cols
    w in [0,W): s = CH*qd - H + w (left halo H, owned CH); z in cols [W, 2W).
    Per step (3 DVE ops, all InstTensorScalarPtr for 2x mode):
      u = sh1(alpha) + alpha ; v = u + sh2(z) ; [alpha|z] = [v,v] * [q|qm]
    Halo refresh every K steps via 3 cross-quadrant copies; periodic
    per-batch renorm by 1/max tracking log corrections.
  - Final: host combines alpha tail, renorm log corrections, and lse sums:
    loss_b = -(log(alpha_T[S-1]+alpha_T[S-2]) + logacc_b - lse_b).
"""

import os
import sys

import numpy as np

sys.path.insert(0, "/opt/trn_rl_repo")

# ---- problem constants (hardcoded per contract) ----
B, T, V, L = 32, 1024, 1024, 100
S = 2 * L + 1  # 201
BLANK = V - 1
N_CORES = 8
BPC = B // N_CORES  # 4 batches per core

# ---- DP layout constants ----
NQ = 4          # S-chunks (one per SBUF quadrant)
CH = 51         # owned states per chunk (NQ*CH = 204 >= S)
K = 16          # steps per halo group
H = 2 * K       # left halo width
W = CH + H      # window cols per chunk (67)
NG = T // K     # 128 groups
TC = 128        # t-chunk size for gather phase
NTC = T // TC   # 8
KR = 32         # renorm period (steps); must be multiple of K
SP = 224        # padded s' extent in staging buffer (s' = s + H)
NVT = V // 128  # 8 v-tiles
SPAD = 208      # padded S for one-hot (2 psum tiles: 128 + 80)
ST1 = S - 128   # rows in second s-tile (73)


def _build_bass(dp_repeat=1):
    import concourse.bacc as bacc
    import concourse.tile as tile
    from concourse import mybir

    f32 = mybir.dt.float32
    nc = bacc.Bacc("TRN2", target_bir_lowering=False)

    logits_in = nc.dram_tensor("logits", [BPC, T, V], f32, kind="ExternalInput")
    eoh_in = nc.dram_tensor("eoh", [BPC, NVT, 128, SPAD], f32, kind="ExternalInput")
    mdp_in = nc.dram_tensor("mdp", [128, W], f32, kind="ExternalInput")
    ident_in = nc.dram_tensor("ident", [128, 128], f32, kind="ExternalInput")
    pdown_in = nc.dram_tensor("pdown", [128, 128], f32, kind="ExternalInput")
    out_alpha = nc.dram_tensor("out_alpha", [4, 2], f32, kind="ExternalOutput")
    out_logacc = nc.dram_tensor("out_logacc", [128, 1], f32, kind="ExternalOutput")
    out_lse = nc.dram_tensor("out_lse", [128, BPC], f32, kind="ExternalOutput")

    AF = mybir.ActivationFunctionType
    OP = mybir.AluOpType

    with tile.TileContext(nc) as tc_:
        import contextlib

        with contextlib.ExitStack() as ctx:
            singles = ctx.enter_context(tc_.tile_pool(name="singles", bufs=1))
            lgp = ctx.enter_context(tc_.tile_pool(name="lgp", bufs=3))
            scrp = ctx.enter_context(tc_.tile_pool(name="scrp", bufs=2))
            ptp = ctx.enter_context(tc_.tile_pool(name="ptp", bufs=2, space="PSUM"))
            pgp = ctx.enter_context(tc_.tile_pool(name="pgp", bufs=2, space="PSUM"))
            php = ctx.enter_context(tc_.tile_pool(name="php", bufs=2, space="PSUM"))
            lgtp = ctx.enter_context(tc_.tile_pool(name="lgtp", bufs=2))
            qsbp = ctx.enter_context(tc_.tile_pool(name="qsbp", bufs=3))
            dramp = ctx.enter_context(tc_.tile_pool(name="dramp", bufs=1, space="DRAM"))

            # staging buffer in DP layout: row 32*qd+b, col w*T + t
            WT = W * T
            qdp = dramp.tile([128, WT], f32, name="qdp", tag="qdp")

            # --- persistent SBUF tiles ---
            e_t = [singles.tile([128, NVT * SPAD], f32, name=f"e{b}", tag=f"e{b}") for b in range(BPC)]
            for b in range(BPC):
                for vt in range(NVT):
                    nc.sync.dma_start(
                        e_t[b][:, vt * SPAD : (vt + 1) * SPAD], eoh_in[b, vt]
                    )
            mdp_t = singles.tile([128, W], f32, tag="mdp")
            nc.sync.dma_start(mdp_t[:], mdp_in[:])
            ident_t = singles.tile([128, 128], f32, tag="ident")
            nc.sync.dma_start(ident_t[:], ident_in[:])
            pdown_t = singles.tile([128, 128], f32, tag="pdown")
            nc.sync.dma_start(pdown_t[:], pdown_in[:])

            zeros_t = singles.tile([128, 1024], f32, tag="zeros")
            nc.vector.memset(zeros_t[:], 0.0)
            # zero staging pads: qd=0 rows, w<H (s<0); qd=3 rows, w>=64 (s>=S)
            for w in range(H):
                nc.sync.dma_start(qdp[0:BPC, w * T : w * T + T], zeros_t[0:BPC, :])
            for w in range(S - 3 * CH + H, W):
                nc.sync.dma_start(
                    qdp[96 : 96 + BPC, w * T : w * T + T], zeros_t[0:BPC, :]
                )

            QGC = 64  # steps per q-load chunk
            qg_ab = [
                singles.tile([128, 2 * W * QGC], f32, name=f"qgab{i}", tag=f"qgab{i}")
                for i in range(2)
            ]
            for i in range(2):
                nc.vector.memset(qg_ab[i][:], 0.0)

            alpha = singles.tile([128, 2 * W], f32, tag="alpha")
            u_t = singles.tile([128, W], f32, tag="u")
            v_t = singles.tile([128, W], f32, tag="v")
            nc.vector.memset(alpha[:], 0.0)
            nc.vector.memset(u_t[:], 0.0)
            nc.vector.memset(v_t[:], 0.0)
            # alpha_init: one-hot at s=0 -> chunk 0 col w=H, batches rows 0..3
            nc.vector.memset(alpha[0:BPC, H : H + 1], 1.0)

            lse_acc = singles.tile([128, BPC], f32, tag="lse")
            nc.vector.memset(lse_acc[:], 0.0)
            logacc = singles.tile([128, 1], f32, tag="logacc")
            nc.vector.memset(logacc[:], 0.0)
            m1_t = singles.tile([128, 1], f32, tag="m1")
            c_t = singles.tile([128, 1], f32, tag="c")
            d_t = singles.tile([128, 1], f32, tag="d")
            ln_t = singles.tile([128, 1], f32, tag="ln")
            lshD_t = singles.tile([128, 1], f32, tag="lshD")
            dlD_t = singles.tile([128, 1], f32, tag="dlD")
            dlpD_t = singles.tile([128, 1], f32, tag="dlpD")
            dlmD_t = singles.tile([128, 1], f32, tag="dlmD")
            fcD_t = singles.tile([128, 1], f32, tag="fcD")
            fdD_t = singles.tile([128, 1], f32, tag="fdD")
            fac_t = singles.tile([128, 1], f32, tag="fac")
            nc.vector.memset(fcD_t[:], 1.0)

            def phase_a(itc):
                # gather + lse for t-chunk itc, all batches
                for b in range(BPC):
                    lg = lgp.tile([128, V], f32, tag="lg")
                    nc.sync.dma_start(lg[:], logits_in[b, itc * TC : (itc + 1) * TC, :])
                    # lse: exp accum -> ln -> accumulate
                    scr = scrp.tile([128, V], f32, tag="scr")
                    se = scrp.tile([128, 1], f32, tag="se")
                    nc.scalar.activation(scr[:], lg[:], AF.Exp, accum_out=se[:])
                    ln1 = scrp.tile([128, 1], f32, tag="ln1")
                    nc.scalar.activation(ln1[:], se[:], AF.Ln)
                    nc.vector.tensor_add(
                        lse_acc[:, b : b + 1], lse_acc[:, b : b + 1], ln1[:]
                    )
                    # transpose to [v, t]
                    lgt = lgtp.tile([128, NVT * 128], f32, tag="lgt")
                    for vt in range(NVT):
                        pt = ptp.tile([128, 128], f32, tag="pt")
                        nc.tensor.transpose(
                            pt[:], lg[:, vt * 128 : (vt + 1) * 128], ident_t[:]
                        )
                        nc.scalar.copy(lgt[:, vt * 128 : (vt + 1) * 128], pt[:])
                    # gather matmuls: out[s_tile, t] += E[vt][:, s_tile].T @ lgt[vt]
                    for st in range(2):
                        srows = 128 if st == 0 else ST1
                        pg = pgp.tile([128, 128], f32, tag="pg")
                        for vt in range(NVT):
                            nc.tensor.matmul(
                                pg[0:srows, :],
                                e_t[b][:, vt * SPAD + st * 128 : vt * SPAD + st * 128 + srows],
                                lgt[:, vt * 128 : (vt + 1) * 128],
                                start=(vt == 0),
                                stop=(vt == NVT - 1),
                            )
                        qsb = qsbp.tile([128, 128], f32, tag="qsb")
                        nc.scalar.activation(qsb[0:srows, :], pg[0:srows, :], AF.Exp)
                        # scatter to DP-layout staging: pieces (part_lo, count, row, w0)
                        if st == 0:
                            pieces = [
                                (0, 51, b, 32),
                                (19, 83, 32 + b, 0),
                                (70, 58, 64 + b, 0),
                                (121, 7, 96 + b, 0),
                            ]
                        else:
                            pieces = [(0, 25, 64 + b, 58), (0, 73, 96 + b, 7)]
                        import concourse.bass as bass
                        for (plo, cnt, row, w0) in pieces:
                            dst = bass.AP(
                                tensor=qdp.tensor,
                                offset=qdp[row : row + 1, w0 * T + itc * TC].offset,
                                ap=[[T, cnt], [1, TC]],
                            )
                            nc.sync.dma_start(dst, qsb[plo : plo + cnt, :])

            import concourse.bass as bass

            def load_qchunk(jc):
                # one plain DMA: qdp rows -> qg tile, cols w*QGC + tau
                qg = qg_ab[jc % 2]
                rl = qg.ap[0][0]
                for qd in range(NQ):
                    src_ap = bass.AP(
                        tensor=qdp.tensor,
                        offset=qdp[32 * qd : 32 * qd + 1, jc * QGC : jc * QGC + 1].offset,
                        ap=[[WT, BPC], [T, W], [1, QGC]],
                    )
                    dst_ap = bass.AP(
                        tensor=qg.tensor,
                        offset=qg[32 * qd : 32 * qd + 1, 0:1].offset,
                        ap=[[rl, BPC], [QGC, W], [1, QGC]],
                    )
                    nc.sync.dma_start(dst_ap, src_ap)
                # qm half: qm = q * m' (broadcast m' over tau) on GpSimd
                q_half = bass.AP(
                    tensor=qg.tensor,
                    offset=qg[0:128, 0:1].offset,
                    ap=[qg.ap[0], [QGC, W], [1, QGC]],
                )
                qm_half = bass.AP(
                    tensor=qg.tensor,
                    offset=qg[0:128, W * QGC : W * QGC + 1].offset,
                    ap=[qg.ap[0], [QGC, W], [1, QGC]],
                )
                m_b = bass.AP(
                    tensor=mdp_t.tensor,
                    offset=mdp_t[0:128, 0:1].offset,
                    ap=[mdp_t.ap[0], [1, W], [0, QGC]],
                )
                nc.gpsimd.tensor_tensor(qm_half, q_half, m_b, OP.mult)
                return qg

            def dp_group(g, qg):
                tau0 = (g * K) % QGC

                if g > 0 and (g * K) % KR == 0:
                    # per-row renorm with overflow-free chunk-scale sync
                    nc.vector.tensor_reduce(
                        m1_t[:], alpha[:, 0:W], mybir.AxisListType.X, OP.max
                    )
                    # d = max(m1, 1e-30); empty rows drift low but the dlp
                    # sync pulls them back up within the same renorm
                    nc.vector.tensor_single_scalar(d_t[:], m1_t[:], 1e-30, OP.max)
                    # logacc += ln(d)  (2^-30 pre-scale keeps ACT Ln in range)
                    nc.scalar.activation(ln_t[:], d_t[:], AF.Ln, scale=2.0**-30)
                    nc.vector.scalar_tensor_tensor(
                        logacc[:], ln_t[:], 30.0 * 0.6931471805599453, logacc[:],
                        OP.add, OP.add,
                    )
                    # lshD[p] = logacc[p-32] via PE shift-matmul (rows<32 -> 0)
                    ps1 = php.tile([128, 1], f32, tag="ps1")
                    nc.tensor.matmul(ps1[:], pdown_t[:], logacc[:])
                    nc.vector.tensor_copy(lshD_t[:], ps1[:])
                    # dlD[p] = logacc[src p-32] - logacc[dst p]
                    nc.vector.tensor_sub(dlD_t[:], lshD_t[:], logacc[:])
                    nc.vector.tensor_scalar_max(dlpD_t[:], dlD_t[:], 0.0)
                    nc.vector.memset(dlpD_t[0:32, :], 0.0)
                    nc.vector.tensor_sub(dlmD_t[:], dlD_t[:], dlpD_t[:])
                    nc.vector.memset(dlmD_t[0:32, :], 0.0)
                    nc.scalar.activation(fcD_t[:], dlmD_t[:], AF.Exp)
                    nc.scalar.activation(fdD_t[:], dlpD_t[:], AF.Exp, scale=-1.0)
                    # rows<32: dlpD=0 -> fdD=1, logacc+=0 (no-ops there)
                    nc.vector.tensor_add(logacc[:], logacc[:], dlpD_t[:])
                    # combined row factor = 1/d * fdD
                    nc.vector.reciprocal(fac_t[:], d_t[:])
                    nc.vector.tensor_mul(fac_t[:], fac_t[:], fdD_t[:])
                    nc.vector.tensor_scalar_mul(alpha[:, :], alpha[:, :], fac_t[:])

                if g > 0:
                    # halo: shift all rows down 32 via PE matmul, then 3
                    # same-partition scaled evacs (factor fcD) into halo cols
                    h_src = bass.AP(
                        tensor=alpha.tensor,
                        offset=alpha[0:128, CH : CH + 1].offset,
                        ap=[alpha.ap[0], [W, 2], [1, H]],
                    )
                    psh = php.tile([128, 2 * H], f32, tag="psh")
                    nc.tensor.matmul(psh[:], pdown_t[:], h_src)
                    h_dst = bass.AP(
                        tensor=alpha.tensor,
                        offset=alpha[0:128, 0:1].offset,
                        ap=[alpha.ap[0], [W, 2], [1, H]],
                    )
                    h_in = bass.AP(
                        tensor=psh.tensor,
                        offset=psh[0:128, 0:1].offset,
                        ap=[psh.ap[0], [H, 2], [1, H]],
                    )
                    nc.scalar.mul(h_dst, h_in, fcD_t[:])

                for j in range(K):
                    tau = tau0 + j
                    # op1: u[2:W] = alpha[1:W-1] + alpha[2:W]
                    nc.vector.scalar_tensor_tensor(
                        u_t[:, 2:W], alpha[:, 1 : W - 1], 0.0, alpha[:, 2:W],
                        OP.add, OP.add,
                    )
                    # op2: v[2:W] = u[2:W] + z[0:W-2]
                    nc.vector.scalar_tensor_tensor(
                        v_t[:, 2:W], u_t[:, 2:W], 0.0, alpha[:, W : 2 * W - 2],
                        OP.add, OP.add,
                    )
                    # op3: [alpha|z][2:W] = [v,v] * [q|qm]_t
                    out_ap = bass.AP(
                        tensor=alpha.tensor,
                        offset=alpha[0:128, 2:3].offset,
                        ap=[alpha.ap[0], [W, 2], [1, W - 2]],
                    )
                    v_dup = bass.AP(
                        tensor=v_t.tensor,
                        offset=v_t[0:128, 2:3].offset,
                        ap=[v_t.ap[0], [0, 2], [1, W - 2]],
                    )
                    q_ap = bass.AP(
                        tensor=qg.tensor,
                        offset=qg[0:128, 2 * QGC + tau : 2 * QGC + tau + 1].offset,
                        ap=[qg.ap[0], [W * QGC, 2], [QGC, W - 2]],
                    )
                    nc.vector.scalar_tensor_tensor(
                        out_ap, v_dup, 1.0, q_ap, OP.mult, OP.mult
                    )

            for itc in range(NTC):
                phase_a(itc)
                for jc in range(2 * itc, 2 * itc + 2):
                    qg = load_qchunk(jc)
                    for g in range(jc * (QGC // K), (jc + 1) * (QGC // K)):
                        for _rep in range(dp_repeat):
                            dp_group(g, qg)

            # final outputs
            # alpha tail: s = S-2, S-1 -> qd=3, w = s - CH*3 + H
            w199 = (S - 2) - CH * 3 + H
            nc.sync.dma_start(out_alpha[:, :], alpha[96:100, w199 : w199 + 2])
            nc.sync.dma_start(out_logacc[:, :], logacc[:])
            nc.sync.dma_start(out_lse[:, :], lse_acc[:])

    nc.compile()
    return nc


def _host_prep(targets_np, logits_np, core):
    """Build per-core input map."""
    bs = core * BPC
    tg = targets_np[bs : bs + BPC]
    ext = np.full((BPC, S), BLANK, dtype=np.int64)
    ext[:, 1::2] = tg
    m = np.zeros((BPC, S), dtype=np.float32)
    m[:, 2:] = ((ext[:, 2:] != BLANK) & (ext[:, 2:] != ext[:, :-2])).astype(np.float32)
    # m'[s] = m[s+2]
    mp = np.zeros((BPC, S), dtype=np.float32)
    mp[:, : S - 2] = m[:, 2:]

    eoh = np.zeros((BPC, NVT, 128, SPAD), dtype=np.float32)
    for b in range(BPC):
        for s in range(S):
            vv = ext[b, s]
            eoh[b, vv // 128, vv % 128, s] = 1.0

    mdp = np.zeros((128, W), dtype=np.float32)
    for qd in range(NQ):
        for b in range(BPC):
            for w in range(W):
                s = CH * qd - H + w
                if 0 <= s < S:
                    mdp[32 * qd + b, w] = mp[b, s]

    ident = np.eye(128, dtype=np.float32)
    pdown = np.zeros((128, 128), dtype=np.float32)
    for mm in range(32, 128):
        pdown[mm - 32, mm] = 1.0
    return {
        "logits": np.ascontiguousarray(logits_np[bs : bs + BPC]).astype(np.float32),
        "eoh": eoh,
        "mdp": mdp,
        "ident": ident,
        "pdown": pdown,
    }


_CACHED_NC = None
_LAST_RESULT = None


def kernel(targets, logits):
    global _CACHED_NC, _LAST_RESULT
    from concourse.bass_utils import run_bass_kernel_spmd

    targets_np = np.asarray(targets)
    logits_np = np.asarray(logits, dtype=np.float32)

    if _CACHED_NC is None:
        _CACHED_NC = _build_bass()
    nc = _CACHED_NC

    in_maps = [_host_prep(targets_np, logits_np, c) for c in range(N_CORES)]
    trace = bool(os.environ.get("CTC_TRACE"))
    res = run_bass_kernel_spmd(
        nc, in_maps, core_ids=list(range(N_CORES)), trace=trace
    )
    _LAST_RESULT = res

    losses = []
    for c in range(N_CORES):
        r = res.results[c]
        a_tail = r["out_alpha"]  # [4, 2]
        logac = r["out_logacc"][:, 0]  # [128], per DP row
        lse = r["out_lse"]  # [128, BPC]
        for b in range(BPC):
            loglik = (
                np.log(a_tail[b, 0] + a_tail[b, 1])
                + logac[96 + b]
                - lse[:, b].sum()
            )
            losses.append(-loglik)
    return np.float32(np.mean(losses))



# revision 6
# speedup vs baseline: 1.5368x; 1.5368x over previous
"""CTC loss kernel for Trainium2 (Bass/Tile), 8-core data-parallel over batch.

Per core (4 batches):
  - gather: ACT exp(logits)->bf16 (accumulates per-t sum-exp for the lse
    path), PE transposes exp'd tiles to [v,t] bf16, then gather matmuls
    with one-hot E[v,s'] (x0.5 prescale baked in) -> PSUM q[t,s'];
    evac + DMA to DRAM staging qdp[b][t][s'] (s' = s+H, zero-padded by
    E's zero columns).
  - DP: probability-domain CTC forward, 1024 serial steps, 2 DVE ops
    per step:  P = [a<<2|a<<1|a] (x) [qm|q|q]  (3-block tensor mult),
    alpha' = segmented-reduce3(P).  State alpha[128,W]: row 16*qd+b
    holds s-chunk qd of batch b, col w <-> s = CH*qd - H + w (left halo
    H, owned CH).  Halo refresh every K steps via a PE shift-16 matmul
    (pdown) with the cross-quadrant scale factor fc applied during the
    PSUM import.  Renorm every KR steps by power-of-2 row scales
    (exponent extracted with int shift ops; no Ln/Exp tables), with
    cross-quadrant scale sync in integer exponent space (eacc ledger,
    pull-up for lagging rows).
  - Final: host combines alpha tail, the integer exponent ledger eacc,
    prescale correction T*ln2, and the deferred-Ln lse sums.
"""

import os
import sys

import numpy as np

sys.path.insert(0, "/opt/trn_rl_repo")

# ---- problem constants (hardcoded per contract) ----
B, T, V, L = 32, 1024, 1024, 100
S = 2 * L + 1  # 201
BLANK = V - 1
N_CORES = 8
BPC = B // N_CORES  # 4 batches per core

# ---- DP layout constants ----
NQ = 8          # s-chunks
CH = 26         # owned states per chunk (NQ*CH = 208 >= 201)
K = 8           # steps per halo group
H = 2 * K       # left halo width (16)
W = CH + H      # window cols per chunk (42)
KR = 64         # renorm period (steps); multiple of K
QGC = 64        # steps per q-load chunk
NCH = T // QGC  # 16
TC = 128        # t-chunk size for gather phase
NTC = T // TC   # 8
NVT = V // 128  # 8 v-tiles
SP = 224        # padded s' extent in staging (s' = s + H)
FD3 = W - 2     # 40
PRE = 0.5       # static q prescale baked into E (loss corr: +T*ln2)


def _row0(qd):
    return 16 * qd


def _build_bass():
    import concourse.bacc as bacc
    import concourse.tile as tile
    from concourse import mybir

    f32 = mybir.dt.float32
    bf16 = mybir.dt.bfloat16
    i32 = mybir.dt.int32
    nc = bacc.Bacc("TRN2", target_bir_lowering=False)

    logits_in = nc.dram_tensor("logits", [BPC, T, V], f32, kind="ExternalInput")
    eoh_in = nc.dram_tensor("eoh", [BPC, NVT, 128, SP], bf16, kind="ExternalInput")
    mdp_in = nc.dram_tensor("mdp", [128, W], f32, kind="ExternalInput")
    ident_in = nc.dram_tensor("ident", [128, 128], bf16, kind="ExternalInput")
    pdown_in = nc.dram_tensor("pdown", [128, 128], f32, kind="ExternalInput")
    cints_in = nc.dram_tensor("cints", [128, 2], i32, kind="ExternalInput")
    out_alpha = nc.dram_tensor("out_alpha", [4, 2], f32, kind="ExternalOutput")
    out_eacc = nc.dram_tensor("out_eacc", [128, 1], f32, kind="ExternalOutput")
    out_lnse = nc.dram_tensor("out_lnse", [128, BPC * NTC], f32, kind="ExternalOutput")

    AF = mybir.ActivationFunctionType
    OP = mybir.AluOpType

    with tile.TileContext(nc) as tc_:
        import contextlib

        import concourse.bass as bass

        with contextlib.ExitStack() as ctx:
            singles = ctx.enter_context(tc_.tile_pool(name="singles", bufs=1))
            lgp = ctx.enter_context(tc_.tile_pool(name="lgp", bufs=2))
            scrp = ctx.enter_context(tc_.tile_pool(name="scrp", bufs=2))
            lgtp = ctx.enter_context(tc_.tile_pool(name="lgtp", bufs=2))
            qsbp = ctx.enter_context(tc_.tile_pool(name="qsbp", bufs=2))
            ptp = ctx.enter_context(tc_.tile_pool(name="ptp", bufs=2, space="PSUM"))
            pgq = ctx.enter_context(tc_.tile_pool(name="pgq", bufs=2, space="PSUM"))
            php = ctx.enter_context(tc_.tile_pool(name="php", bufs=2, space="PSUM"))
            dramp = ctx.enter_context(tc_.tile_pool(name="dramp", bufs=1, space="DRAM"))

            # staging buffer: [b][t][s'] fp32
            qdp = dramp.tile([BPC, T * SP], f32, name="qdp", tag="qdp")

            # --- persistent SBUF tiles ---
            e_t = [
                singles.tile([128, NVT * SP], bf16, name=f"e{b}", tag=f"e{b}")
                for b in range(BPC)
            ]
            for b in range(BPC):
                for vt in range(NVT):
                    nc.sync.dma_start(
                        e_t[b][:, vt * SP : (vt + 1) * SP], eoh_in[b, vt]
                    )
            mdp_t = singles.tile([128, W], f32, tag="mdp")
            nc.sync.dma_start(mdp_t[:], mdp_in[:])
            ident_t = singles.tile([128, 128], bf16, tag="ident")
            nc.sync.dma_start(ident_t[:], ident_in[:])
            pdown_t = singles.tile([128, 128], f32, tag="pdown")
            nc.sync.dma_start(pdown_t[:], pdown_in[:])
            cints_t = singles.tile([128, 2], i32, tag="cints")
            nc.sync.dma_start(cints_t[:], cints_in[:])

            alpha = singles.tile([128, W], f32, tag="alpha")
            P_t = singles.tile([128, 3 * FD3], f32, tag="P")
            nc.vector.memset(alpha[:], 0.0)
            nc.vector.memset(P_t[:], 0.0)
            # alpha init: delta at s=0 -> qd 0 rows 0..3, col w=H
            nc.vector.memset(alpha[0:BPC, H : H + 1], 1.0)

            qg_ab = [
                singles.tile([128, QGC * 3 * W], f32, name=f"qgab{i}", tag=f"qgab{i}")
                for i in range(2)
            ]
            for i in range(2):
                nc.vector.memset(qg_ab[i][:], 0.0)

            seall = singles.tile([128, BPC * NTC], f32, tag="seall")
            lnse_t = singles.tile([128, BPC * NTC], f32, tag="lnse")

            # renorm tiles (power-of-2 exponent ledger)
            me_t = singles.tile([128, 1], f32, tag="me")
            beI_t = singles.tile([128, 1], i32, tag="beI")
            ef_t = singles.tile([128, 1], f32, tag="ef")
            eacc_t = singles.tile([128, 1], f32, tag="eacc")
            dl_t = singles.tile([128, 1], f32, tag="dl")
            dlp_t = singles.tile([128, 1], f32, tag="dlp")
            dlm_t = singles.tile([128, 1], f32, tag="dlm")
            dlpI_t = singles.tile([128, 1], i32, tag="dlpI")
            dlmI_t = singles.tile([128, 1], i32, tag="dlmI")
            sI_t = singles.tile([128, 1], i32, tag="sI")
            fI_t = singles.tile([128, 1], i32, tag="fI")
            sc_t = singles.tile([128, 1], f32, tag="sc")
            fc_t = singles.tile([128, 1], f32, tag="fc")
            c126_t = singles.tile([128, 1], f32, tag="c126")
            cn126_t = singles.tile([128, 1], f32, tag="cn126")
            nc.vector.memset(eacc_t[:], 0.0)
            nc.vector.memset(fc_t[:], 1.0)
            nc.vector.memset(c126_t[:], 126.0)
            nc.vector.memset(cn126_t[:], -126.0)

            def phase_a(itc):
                for b in range(BPC):
                    lg = lgp.tile([128, V], f32, tag="lg")
                    nc.sync.dma_start(lg[:], logits_in[b, itc * TC : (itc + 1) * TC, :])
                    scr = scrp.tile([128, V], bf16, tag="scr")
                    nc.scalar.activation(
                        scr[:], lg[:], AF.Exp,
                        accum_out=seall[:, b * NTC + itc : b * NTC + itc + 1],
                    )
                    lgt = lgtp.tile([128, NVT * 128], bf16, tag="lgt")
                    for vt in range(NVT):
                        pt = ptp.tile([128, 128], bf16, tag="pt")
                        nc.tensor.transpose(
                            pt[:], scr[:, vt * 128 : (vt + 1) * 128], ident_t[:]
                        )
                        nc.scalar.copy(lgt[:, vt * 128 : (vt + 1) * 128], pt[:])
                    pq = pgq.tile([128, SP], f32, tag="pq")
                    for vt in range(NVT):
                        nc.tensor.matmul(
                            pq[:],
                            lgt[:, vt * 128 : (vt + 1) * 128],
                            e_t[b][:, vt * SP : (vt + 1) * SP],
                            start=(vt == 0),
                            stop=(vt == NVT - 1),
                        )
                    qsb = qsbp.tile([128, SP], f32, tag="qsb")
                    nc.scalar.copy(qsb[:], pq[:])
                    dst = bass.AP(
                        tensor=qdp.tensor,
                        offset=qdp[b : b + 1, itc * TC * SP].offset,
                        ap=[[SP, TC], [1, SP]],
                    )
                    nc.sync.dma_start(dst, qsb[:])

            def load_q(jc):
                qg = qg_ab[jc % 2]
                rl = qg.ap[0][0]
                for qd in range(NQ):
                    r0 = _row0(qd)
                    for d in (1, 2):
                        src_ap = bass.AP(
                            tensor=qdp.tensor,
                            offset=qdp[0:1, jc * QGC * SP + CH * qd].offset,
                            ap=[[T * SP, BPC], [SP, QGC], [1, W]],
                        )
                        dst_ap = bass.AP(
                            tensor=qg.tensor,
                            offset=qg[r0 : r0 + 1, d * W].offset,
                            ap=[[rl, BPC], [3 * W, QGC], [1, W]],
                        )
                        nc.sync.dma_start(dst_ap, src_ap)
                # qm block: qm = q * mdp (broadcast mdp over tau) on GpSimd
                qm_ap = bass.AP(
                    tensor=qg.tensor,
                    offset=qg[0:128, 0:1].offset,
                    ap=[qg.ap[0], [3 * W, QGC], [1, W]],
                )
                q_ap = bass.AP(
                    tensor=qg.tensor,
                    offset=qg[0:128, W : W + 1].offset,
                    ap=[qg.ap[0], [3 * W, QGC], [1, W]],
                )
                m_b = bass.AP(
                    tensor=mdp_t.tensor,
                    offset=mdp_t[0:128, 0:1].offset,
                    ap=[mdp_t.ap[0], [0, QGC], [1, W]],
                )
                nc.gpsimd.tensor_tensor(qm_ap, q_ap, m_b, OP.mult)

            def renorm():
                nc.vector.tensor_reduce(
                    me_t[:], alpha[:, 0:W], mybir.AxisListType.X, OP.max
                )
                nc.vector.tensor_single_scalar(
                    beI_t[:], me_t[:].bitcast(i32), 23, OP.logical_shift_right
                )
                nc.vector.tensor_copy(ef_t[:], beI_t[:])
                nc.vector.scalar_tensor_tensor(
                    eacc_t[:], ef_t[:], -127.0, eacc_t[:], OP.add, OP.add
                )
                # esrc[p] = eacc[p-16] via PE shift matmul (rows<16 -> 0)
                psr = php.tile([128, 1], f32, tag="psr")
                nc.tensor.matmul(psr[:], pdown_t[:], eacc_t[:])
                nc.vector.tensor_sub(dl_t[:], psr[:], eacc_t[:])
                nc.vector.memset(dl_t[0:16, :], 0.0)
                # dlp = clamp(dl, 0, 126); dlm = clamp(dl, -126, 0)
                nc.vector.scalar_tensor_tensor(
                    dlp_t[:], dl_t[:], 0.0, c126_t[:], OP.max, OP.min
                )
                nc.vector.scalar_tensor_tensor(
                    dlm_t[:], dl_t[:], 0.0, cn126_t[:], OP.min, OP.max
                )
                nc.vector.tensor_add(eacc_t[:], eacc_t[:], dlp_t[:])
                # scale bits = (254 - be - dlp) << 23  -> sc = 2^(-e-dlp)
                nc.vector.tensor_copy(dlpI_t[:], dlp_t[:])
                nc.vector.tensor_sub(sI_t[:], cints_t[:, 0:1], beI_t[:])
                nc.vector.tensor_sub(sI_t[:], sI_t[:], dlpI_t[:])
                nc.vector.tensor_single_scalar(sI_t[:], sI_t[:], 0, OP.max)
                nc.vector.tensor_single_scalar(
                    sc_t[:].bitcast(i32), sI_t[:], 23, OP.logical_shift_left
                )
                # fc bits = (dlm + 127) << 23 -> fc = 2^dlm
                nc.vector.tensor_copy(dlmI_t[:], dlm_t[:])
                nc.vector.tensor_add(fI_t[:], dlmI_t[:], cints_t[:, 1:2])
                nc.vector.tensor_single_scalar(
                    fc_t[:].bitcast(i32), fI_t[:], 23, OP.logical_shift_left
                )
                nc.vector.tensor_scalar_mul(alpha[:, :], alpha[:, :], sc_t[:])

            def dp_group(gg, qg):
                tau0 = (gg * K) % QGC
                if gg > 0:
                    if (gg * K) % KR == 0:
                        renorm()
                    # halo: rows p cols [0,H) <- rows p-16 cols [CH,CH+H) * fc
                    psh = php.tile([128, H], f32, tag="psh")
                    nc.tensor.matmul(psh[:], pdown_t[:], alpha[:, CH : CH + H])
                    nc.vector.tensor_scalar_mul(alpha[:, 0:H], psh[:], fc_t[:])

                for j in range(K):
                    tau = tau0 + j
                    p_ap = bass.AP(
                        tensor=P_t.tensor,
                        offset=P_t[0:128, 0:1].offset,
                        ap=[P_t.ap[0], [FD3, 3], [1, FD3]],
                    )
                    a_ap = bass.AP(
                        tensor=alpha.tensor,
                        offset=alpha[0:128, 0:1].offset,
                        ap=[alpha.ap[0], [1, 3], [1, FD3]],
                    )
                    q_ap = bass.AP(
                        tensor=qg.tensor,
                        offset=qg[0:128, tau * 3 * W + 2].offset,
                        ap=[qg.ap[0], [W, 3], [1, FD3]],
                    )
                    nc.vector.tensor_mul(p_ap, a_ap, q_ap)
                    pr_ap = bass.AP(
                        tensor=P_t.tensor,
                        offset=P_t[0:128, 0:1].offset,
                        ap=[P_t.ap[0], [1, FD3], [FD3, 3]],
                    )
                    nc.vector.tensor_reduce(
                        alpha[:, 2:W], pr_ap, mybir.AxisListType.X, OP.add
                    )

            # ---- main pipeline ----
            phase_a(0)
            load_q(0)
            for jc in range(NCH):
                if jc % 2 == 0 and jc // 2 + 1 < NTC:
                    phase_a(jc // 2 + 1)
                if jc + 1 < NCH:
                    load_q(jc + 1)
                for gg in range(jc * (QGC // K), (jc + 1) * (QGC // K)):
                    dp_group(gg, qg_ab[jc % 2])

            # final outputs: s = 199, 200 -> qd 7 rows 112..115, w = 33
            w199 = (S - 2) - CH * 7 + H
            nc.sync.dma_start(out_alpha[:, :], alpha[112:116, w199 : w199 + 2])
            nc.sync.dma_start(out_eacc[:, :], eacc_t[:])
            nc.scalar.activation(lnse_t[:], seall[:], AF.Ln)
            nc.sync.dma_start(out_lnse[:, :], lnse_t[:])

    nc.compile()
    return nc


def _host_prep(targets_np, logits_np, core):
    import ml_dtypes

    bs = core * BPC
    tg = targets_np[bs : bs + BPC]
    ext = np.full((BPC, S), BLANK, dtype=np.int64)
    ext[:, 1::2] = tg
    m = np.zeros((BPC, S), dtype=np.float32)
    m[:, 2:] = ((ext[:, 2:] != BLANK) & (ext[:, 2:] != ext[:, :-2])).astype(np.float32)

    # one-hot E[v, s'] with PRE baked in; s' = s + H, zero cols elsewhere
    eoh = np.zeros((BPC, NVT, 128, SP), dtype=np.float32)
    for b in range(BPC):
        for s in range(S):
            vv = ext[b, s]
            eoh[b, vv // 128, vv % 128, s + H] = PRE

    # mask plane in DP layout: row 16*qd+b, col w <-> s = CH*qd - H + w
    mdp = np.zeros((128, W), dtype=np.float32)
    for qd in range(NQ):
        for b in range(BPC):
            for w in range(W):
                s = CH * qd - H + w
                if 0 <= s < S:
                    mdp[_row0(qd) + b, w] = m[b, s]

    ident = np.eye(128, dtype=np.float32)
    pdown = np.zeros((128, 128), dtype=np.float32)
    for p in range(16, 128):
        pdown[p - 16, p] = 1.0
    cints = np.zeros((128, 2), dtype=np.int32)
    cints[:, 0] = 254
    cints[:, 1] = 127
    return {
        "logits": np.ascontiguousarray(logits_np[bs : bs + BPC]).astype(np.float32),
        "eoh": eoh.astype(ml_dtypes.bfloat16),
        "mdp": mdp,
        "ident": ident.astype(ml_dtypes.bfloat16),
        "pdown": pdown,
        "cints": cints,
    }


_CACHED_NC = None
_LAST_RESULT = None


def kernel(targets, logits):
    global _CACHED_NC, _LAST_RESULT
    from concourse.bass_utils import run_bass_kernel_spmd

    targets_np = np.asarray(targets)
    logits_np = np.asarray(logits, dtype=np.float32)

    if _CACHED_NC is None:
        _CACHED_NC = _build_bass()
    nc = _CACHED_NC

    in_maps = [_host_prep(targets_np, logits_np, c) for c in range(N_CORES)]
    trace = bool(os.environ.get("CTC_TRACE"))
    res = run_bass_kernel_spmd(
        nc, in_maps, core_ids=list(range(N_CORES)), trace=trace
    )
    _LAST_RESULT = res

    ln2 = float(np.log(2.0))
    losses = []
    for c in range(N_CORES):
        r = res.results[c]
        a_tail = np.asarray(r["out_alpha"], dtype=np.float64)  # [4, 2]
        eacc = np.asarray(r["out_eacc"], dtype=np.float64)[:, 0]  # [128]
        lnse = np.asarray(r["out_lnse"], dtype=np.float64)  # [128, 32]
        for b in range(BPC):
            lse_b = lnse[:, b * NTC : (b + 1) * NTC].sum()
            loglik = (
                np.log(a_tail[b, 0] + a_tail[b, 1])
                + eacc[112 + b] * ln2
                + T * ln2  # PRE = 0.5 correction
                - lse_b
            )
            losses.append(-loglik)
    return np.float32(np.mean(losses))


# revision 11
# speedup vs baseline: 1.5662x; 1.0191x over previous
"""CTC loss kernel for Trainium2 (Bass/Tile), 8-core data-parallel over batch.

Per core (4 batches):
  - gather: ACT exp(logits)->bf16 (accumulates per-t sum-exp for the lse
    path), PE transposes exp'd tiles to [v,t] bf16, then two gather
    matmul accumulations with one-hot E[v,s'] and mask-baked E2[v,s']
    (x0.5 prescale baked in) -> PSUM q[t,s'] and qm[t,s']; ACT evacs
    into pre-windowed [qd][qm|q|q][w] rows, one DMA to DRAM staging
    qdp[b][t][qd*3W], one readback DMA per (chunk, qd) into the DP
    layout.
  - DP: probability-domain CTC forward, 1024 serial steps, 2 DVE ops
    per step:  P = [a<<2|a<<1|a] (x) [qm|q|q]  (3-block tensor mult),
    alpha' = segmented-reduce3(P).  State alpha[128,W]: row 16*qd+b
    holds s-chunk qd of batch b, col w <-> s = CH*qd - H + w (left halo
    H, owned CH).  Halo refresh every K steps via a PE shift-16 matmul
    (pdown) with the cross-quadrant scale factor fc applied during the
    PSUM import.  Renorm every KR steps by power-of-2 row scales
    (exponent extracted with int shift ops; no Ln/Exp tables), with
    cross-quadrant scale sync in integer exponent space (eacc ledger,
    pull-up for lagging rows).
  - Final: host combines alpha tail, the integer exponent ledger eacc,
    prescale correction T*ln2, and the deferred-Ln lse sums.
"""

import os
import sys

import numpy as np

sys.path.insert(0, "/opt/trn_rl_repo")

# ---- problem constants (hardcoded per contract) ----
B, T, V, L = 32, 1024, 1024, 100
S = 2 * L + 1  # 201
BLANK = V - 1
N_CORES = 8
BPC = B // N_CORES  # 4 batches per core

# ---- DP layout constants ----
NQ = 8          # s-chunks
CH = 26         # owned states per chunk (NQ*CH = 208 >= 201)
K = 8           # steps per halo group
H = 2 * K       # left halo width (16)
W = CH + H      # window cols per chunk (42)
KR = 64         # renorm period (steps); multiple of K
QGC = 64        # steps per q-load chunk
NCH = T // QGC  # 16
TC = 128        # t-chunk size for gather phase
NTC = T // TC   # 8
NVT = V // 128  # 8 v-tiles
SP = 224        # padded s' extent (s' = s + H)
W3 = 3 * W      # per-quadrant staging row: [qm|q|q]
SROW = NQ * W3  # 1008 staging cols per t
FD3 = W - 2     # 40
PRE = 0.5       # static q prescale baked into E (loss corr: +T*ln2)


def _row0(qd):
    return 16 * qd


def _build_bass():
    import concourse.bacc as bacc
    import concourse.tile as tile
    from concourse import mybir

    f32 = mybir.dt.float32
    bf16 = mybir.dt.bfloat16
    i32 = mybir.dt.int32
    nc = bacc.Bacc("TRN2", target_bir_lowering=False)

    logits_in = nc.dram_tensor("logits", [BPC, T, V], f32, kind="ExternalInput")
    eoh_in = nc.dram_tensor("eoh", [BPC, NVT, 128, SP], bf16, kind="ExternalInput")
    eoh2_in = nc.dram_tensor("eoh2", [BPC, NVT, 128, SP], bf16, kind="ExternalInput")
    ident_in = nc.dram_tensor("ident", [128, 128], bf16, kind="ExternalInput")
    pdown_in = nc.dram_tensor("pdown", [128, 128], f32, kind="ExternalInput")
    cints_in = nc.dram_tensor("cints", [128, 2], i32, kind="ExternalInput")
    out_alpha = nc.dram_tensor("out_alpha", [4, 2], f32, kind="ExternalOutput")
    out_eacc = nc.dram_tensor("out_eacc", [128, 1], f32, kind="ExternalOutput")
    out_lnse = nc.dram_tensor("out_lnse", [128, BPC * NTC], f32, kind="ExternalOutput")

    AF = mybir.ActivationFunctionType
    OP = mybir.AluOpType

    with tile.TileContext(nc) as tc_:
        import contextlib

        import concourse.bass as bass

        with contextlib.ExitStack() as ctx:
            singles = ctx.enter_context(tc_.tile_pool(name="singles", bufs=1))
            lgp = ctx.enter_context(tc_.tile_pool(name="lgp", bufs=4))
            scrp = ctx.enter_context(tc_.tile_pool(name="scrp", bufs=2))
            lgtp = ctx.enter_context(tc_.tile_pool(name="lgtp", bufs=2))
            qsbp = ctx.enter_context(tc_.tile_pool(name="qsbp", bufs=2))
            ptp = ctx.enter_context(tc_.tile_pool(name="ptp", bufs=2, space="PSUM"))
            pgq = ctx.enter_context(tc_.tile_pool(name="pgq", bufs=1, space="PSUM"))
            php = ctx.enter_context(tc_.tile_pool(name="php", bufs=1, space="PSUM"))
            dramp = ctx.enter_context(tc_.tile_pool(name="dramp", bufs=1, space="DRAM"))

            # staging buffer: [b][t][qd*3W + blk*W + w] fp32 (pre-windowed)
            qdp = dramp.tile([BPC, T * SROW], f32, name="qdp", tag="qdp")

            # --- first: logits loads for t-chunk 0 (DMA priority) ---
            lg0 = [lgp.tile([128, V], f32, name=f"lg0{b}", tag="lg") for b in range(BPC)]
            for b in range(BPC):
                nc.sync.dma_start(lg0[b][:], logits_in[b, 0:TC, :])

            # --- persistent SBUF tiles ---
            e_t = [
                singles.tile([128, NVT * SP], bf16, name=f"e{b}", tag=f"e{b}")
                for b in range(BPC)
            ]
            e2_t = [
                singles.tile([128, NVT * SP], bf16, name=f"e2{b}", tag=f"e2{b}")
                for b in range(BPC)
            ]
            for b in range(BPC):
                for eo, ei in ((e_t[b], eoh_in), (e2_t[b], eoh2_in)):
                    base = ei[b, 0]
                    src_ap = bass.AP(
                        tensor=base.tensor,
                        offset=base.offset,
                        ap=[[SP, 128], [128 * SP, NVT], [1, SP]],
                    )
                    dst_ap = bass.AP(
                        tensor=eo.tensor,
                        offset=eo[0:1, 0:1].offset,
                        ap=[[eo.ap[0][0], 128], [SP, NVT], [1, SP]],
                    )
                    nc.sync.dma_start(dst_ap, src_ap)
            ident_t = singles.tile([128, 128], bf16, tag="ident")
            nc.sync.dma_start(ident_t[:], ident_in[:])
            pdown_t = singles.tile([128, 128], f32, tag="pdown")
            nc.sync.dma_start(pdown_t[:], pdown_in[:])
            cints_t = singles.tile([128, 2], i32, tag="cints")
            nc.sync.dma_start(cints_t[:], cints_in[:])

            alpha = singles.tile([128, W], f32, tag="alpha")
            P_t = singles.tile([128, 3 * FD3], f32, tag="P")
            nc.vector.memset(alpha[:], 0.0)
            nc.vector.memset(P_t[:], 0.0)
            # alpha init: delta at s=0 -> qd 0 rows 0..3, col w=H
            nc.vector.memset(alpha[0:BPC, H : H + 1], 1.0)

            qg_ab = [
                singles.tile([128, QGC * W3], f32, name=f"qgab{i}", tag=f"qgab{i}")
                for i in range(2)
            ]
            for i in range(2):
                nc.vector.memset(qg_ab[i][:], 0.0)

            seall = singles.tile([128, BPC * NTC], f32, tag="seall")
            lnse_t = singles.tile([128, BPC * NTC], f32, tag="lnse")

            # renorm tiles (power-of-2 exponent ledger)
            me_t = singles.tile([128, 1], f32, tag="me")
            beI_t = singles.tile([128, 1], i32, tag="beI")
            ef_t = singles.tile([128, 1], f32, tag="ef")
            eacc_t = singles.tile([128, 1], f32, tag="eacc")
            dl_t = singles.tile([128, 1], f32, tag="dl")
            dlp_t = singles.tile([128, 1], f32, tag="dlp")
            dlm_t = singles.tile([128, 1], f32, tag="dlm")
            dlpI_t = singles.tile([128, 1], i32, tag="dlpI")
            dlmI_t = singles.tile([128, 1], i32, tag="dlmI")
            sI_t = singles.tile([128, 1], i32, tag="sI")
            fI_t = singles.tile([128, 1], i32, tag="fI")
            sc_t = singles.tile([128, 1], f32, tag="sc")
            fc_t = singles.tile([128, 1], f32, tag="fc")
            c126_t = singles.tile([128, 1], f32, tag="c126")
            cn126_t = singles.tile([128, 1], f32, tag="cn126")
            nc.vector.memset(eacc_t[:], 0.0)
            nc.vector.memset(fc_t[:], 1.0)
            nc.vector.memset(c126_t[:], 126.0)
            nc.vector.memset(cn126_t[:], -126.0)

            def pa_load(itc, b):
                lg = lgp.tile([128, V], f32, tag="lg")
                nc.sync.dma_start(lg[:], logits_in[b, itc * TC : (itc + 1) * TC, :])
                return lg

            def pa_comp(itc, b, lg):
                scr = scrp.tile([128, V], bf16, tag="scr")
                nc.scalar.activation(
                    scr[:], lg[:], AF.Exp,
                    accum_out=seall[:, b * NTC + itc : b * NTC + itc + 1],
                )
                lgt = lgtp.tile([128, NVT * 128], bf16, tag="lgt")
                for vt in range(NVT):
                    pt = ptp.tile([128, 128], bf16, tag="pt")
                    nc.tensor.transpose(
                        pt[:], scr[:, vt * 128 : (vt + 1) * 128], ident_t[:]
                    )
                    nc.scalar.copy(lgt[:, vt * 128 : (vt + 1) * 128], pt[:])
                pq = pgq.tile([128, SP], f32, tag="pq")
                for vt in range(NVT):
                    nc.tensor.matmul(
                        pq[:],
                        lgt[:, vt * 128 : (vt + 1) * 128],
                        e_t[b][:, vt * SP : (vt + 1) * SP],
                        start=(vt == 0),
                        stop=(vt == NVT - 1),
                    )
                pqm = pgq.tile([128, SP], f32, tag="pqm")
                for vt in range(NVT):
                    nc.tensor.matmul(
                        pqm[:],
                        lgt[:, vt * 128 : (vt + 1) * 128],
                        e2_t[b][:, vt * SP : (vt + 1) * SP],
                        start=(vt == 0),
                        stop=(vt == NVT - 1),
                    )
                # evac into pre-windowed [qd][qm|q|q][w] staging rows
                qsb = qsbp.tile([128, SROW], f32, tag="qsb")
                qm_out = bass.AP(
                    tensor=qsb.tensor,
                    offset=qsb[0:1, 0:1].offset,
                    ap=[qsb.ap[0], [W3, NQ], [1, W]],
                )
                qm_in = bass.AP(
                    tensor=pqm.tensor,
                    offset=pqm[0:1, 0:1].offset,
                    ap=[pqm.ap[0], [CH, NQ], [1, W]],
                )
                nc.scalar.copy(qm_out, qm_in)
                for d in (1, 2):
                    q_out = bass.AP(
                        tensor=qsb.tensor,
                        offset=qsb[0:1, d * W].offset,
                        ap=[qsb.ap[0], [W3, NQ], [1, W]],
                    )
                    q_in = bass.AP(
                        tensor=pq.tensor,
                        offset=pq[0:1, 0:1].offset,
                        ap=[pq.ap[0], [CH, NQ], [1, W]],
                    )
                    nc.scalar.copy(q_out, q_in)
                dst = bass.AP(
                    tensor=qdp.tensor,
                    offset=qdp[b : b + 1, itc * TC * SROW].offset,
                    ap=[[SROW, TC], [1, SROW]],
                )
                nc.sync.dma_start(dst, qsb[:])

            def load_q(jc):
                qg = qg_ab[jc % 2]
                rl = qg.ap[0][0]
                for qd in range(NQ):
                    r0 = _row0(qd)
                    src_ap = bass.AP(
                        tensor=qdp.tensor,
                        offset=qdp[0:1, jc * QGC * SROW + qd * W3].offset,
                        ap=[[T * SROW, BPC], [SROW, QGC], [1, W3]],
                    )
                    dst_ap = bass.AP(
                        tensor=qg.tensor,
                        offset=qg[r0 : r0 + 1, 0:1].offset,
                        ap=[[rl, BPC], [W3, QGC], [1, W3]],
                    )
                    nc.sync.dma_start(dst_ap, src_ap)

            def renorm():
                nc.vector.tensor_reduce(
                    me_t[:], alpha[:, 0:W], mybir.AxisListType.X, OP.max
                )
                nc.vector.tensor_single_scalar(
                    beI_t[:], me_t[:].bitcast(i32), 23, OP.logical_shift_right
                )
                nc.vector.tensor_copy(ef_t[:], beI_t[:])
                nc.vector.scalar_tensor_tensor(
                    eacc_t[:], ef_t[:], -127.0, eacc_t[:], OP.add, OP.add
                )
                # esrc[p] = eacc[p-16] via PE shift matmul (rows<16 -> 0)
                psr = php.tile([128, 1], f32, tag="psr")
                nc.tensor.matmul(psr[:], pdown_t[:], eacc_t[:])
                nc.vector.tensor_sub(dl_t[:], psr[:], eacc_t[:])
                nc.vector.memset(dl_t[0:16, :], 0.0)
                # dlp = clamp(dl, 0, 126); dlm = clamp(dl, -126, 0)
                nc.vector.scalar_tensor_tensor(
                    dlp_t[:], dl_t[:], 0.0, c126_t[:], OP.max, OP.min
                )
                nc.vector.scalar_tensor_tensor(
                    dlm_t[:], dl_t[:], 0.0, cn126_t[:], OP.min, OP.max
                )
                nc.vector.tensor_add(eacc_t[:], eacc_t[:], dlp_t[:])
                # scale bits = max(254 - be - dlp, 0) << 23 -> sc = 2^(-e-dlp)
                nc.vector.tensor_copy(dlpI_t[:], dlp_t[:])
                nc.vector.tensor_sub(sI_t[:], cints_t[:, 0:1], beI_t[:])
                nc.vector.tensor_sub(sI_t[:], sI_t[:], dlpI_t[:])
                nc.vector.tensor_single_scalar(sI_t[:], sI_t[:], 0, OP.max)
                nc.vector.tensor_single_scalar(
                    sc_t[:].bitcast(i32), sI_t[:], 23, OP.logical_shift_left
                )
                # fc bits = (dlm + 127) << 23 -> fc = 2^dlm
                nc.vector.tensor_copy(dlmI_t[:], dlm_t[:])
                nc.vector.tensor_add(fI_t[:], dlmI_t[:], cints_t[:, 1:2])
                nc.vector.tensor_single_scalar(
                    fc_t[:].bitcast(i32), fI_t[:], 23, OP.logical_shift_left
                )
                nc.vector.tensor_scalar_mul(alpha[:, :], alpha[:, :], sc_t[:])

            def dp_group(gg, qg):
                tau0 = (gg * K) % QGC
                if gg > 0:
                    if (gg * K) % KR == 0:
                        renorm()
                    # halo: rows p cols [0,H) <- rows p-16 cols [CH,CH+H) * fc
                    psh = php.tile([128, H], f32, tag="psh")
                    nc.tensor.matmul(psh[:], pdown_t[:], alpha[:, CH : CH + H])
                    nc.vector.tensor_scalar_mul(alpha[:, 0:H], psh[:], fc_t[:])

                for j in range(K):
                    tau = tau0 + j
                    p_ap = bass.AP(
                        tensor=P_t.tensor,
                        offset=P_t[0:128, 0:1].offset,
                        ap=[P_t.ap[0], [FD3, 3], [1, FD3]],
                    )
                    a_ap = bass.AP(
                        tensor=alpha.tensor,
                        offset=alpha[0:128, 0:1].offset,
                        ap=[alpha.ap[0], [1, 3], [1, FD3]],
                    )
                    q_ap = bass.AP(
                        tensor=qg.tensor,
                        offset=qg[0:128, tau * W3 + 2].offset,
                        ap=[qg.ap[0], [W, 3], [1, FD3]],
                    )
                    nc.vector.tensor_mul(p_ap, a_ap, q_ap)
                    pr_ap = bass.AP(
                        tensor=P_t.tensor,
                        offset=P_t[0:128, 0:1].offset,
                        ap=[P_t.ap[0], [1, FD3], [FD3, 3]],
                    )
                    nc.vector.tensor_reduce(
                        alpha[:, 2:W], pr_ap, mybir.AxisListType.X, OP.add
                    )

            # ---- main pipeline ----
            for b in range(BPC):
                pa_comp(0, b, lg0[b])
            load_q(0)
            for jc in range(NCH):
                itc_next = jc // 2 + 1
                lg_pend = None
                for gi, gg in enumerate(
                    range(jc * (QGC // K), (jc + 1) * (QGC // K))
                ):
                    if jc % 2 == 0 and itc_next < NTC:
                        if 0 <= gi <= 3:
                            lg_new = pa_load(itc_next, gi)
                            if lg_pend is not None:
                                pa_comp(itc_next, gi - 1, lg_pend)
                            lg_pend = lg_new
                        elif gi == 4:
                            pa_comp(itc_next, 3, lg_pend)
                            lg_pend = None
                    if gi == 6 and jc + 1 < NCH:
                        load_q(jc + 1)
                    dp_group(gg, qg_ab[jc % 2])

            # final outputs: s = 199, 200 -> qd 7 rows 112..115, w = 33
            w199 = (S - 2) - CH * 7 + H
            nc.sync.dma_start(out_alpha[:, :], alpha[112:116, w199 : w199 + 2])
            nc.sync.dma_start(out_eacc[:, :], eacc_t[:])
            nc.scalar.activation(lnse_t[:], seall[:], AF.Ln)
            nc.sync.dma_start(out_lnse[:, :], lnse_t[:])

    nc.compile()
    return nc


def _host_prep(targets_np, logits_np, core):
    import ml_dtypes

    bs = core * BPC
    tg = targets_np[bs : bs + BPC]
    ext = np.full((BPC, S), BLANK, dtype=np.int64)
    ext[:, 1::2] = tg
    m = np.zeros((BPC, S), dtype=np.float32)
    m[:, 2:] = ((ext[:, 2:] != BLANK) & (ext[:, 2:] != ext[:, :-2])).astype(np.float32)

    # one-hot E[v, s'] (and mask-baked E2) with PRE; s' = s + H
    eoh = np.zeros((BPC, NVT, 128, SP), dtype=np.float32)
    eoh2 = np.zeros((BPC, NVT, 128, SP), dtype=np.float32)
    for b in range(BPC):
        for s in range(S):
            vv = ext[b, s]
            eoh[b, vv // 128, vv % 128, s + H] = PRE
            eoh2[b, vv // 128, vv % 128, s + H] = PRE * m[b, s]

    ident = np.eye(128, dtype=np.float32)
    pdown = np.zeros((128, 128), dtype=np.float32)
    for p in range(16, 128):
        pdown[p - 16, p] = 1.0
    cints = np.zeros((128, 2), dtype=np.int32)
    cints[:, 0] = 254
    cints[:, 1] = 127
    return {
        "logits": np.ascontiguousarray(logits_np[bs : bs + BPC]).astype(np.float32),
        "eoh": eoh.astype(ml_dtypes.bfloat16),
        "eoh2": eoh2.astype(ml_dtypes.bfloat16),
        "ident": ident.astype(ml_dtypes.bfloat16),
        "pdown": pdown,
        "cints": cints,
    }


_CACHED_NC = None
_LAST_RESULT = None


def kernel(targets, logits):
    global _CACHED_NC, _LAST_RESULT
    from concourse.bass_utils import run_bass_kernel_spmd

    targets_np = np.asarray(targets)
    logits_np = np.asarray(logits, dtype=np.float32)

    if _CACHED_NC is None:
        _CACHED_NC = _build_bass()
    nc = _CACHED_NC

    in_maps = [_host_prep(targets_np, logits_np, c) for c in range(N_CORES)]
    trace = bool(os.environ.get("CTC_TRACE"))
    res = run_bass_kernel_spmd(
        nc, in_maps, core_ids=list(range(N_CORES)), trace=trace
    )
    _LAST_RESULT = res

    ln2 = float(np.log(2.0))
    losses = []
    for c in range(N_CORES):
        r = res.results[c]
        a_tail = np.asarray(r["out_alpha"], dtype=np.float64)  # [4, 2]
        eacc = np.asarray(r["out_eacc"], dtype=np.float64)[:, 0]  # [128]
        lnse = np.asarray(r["out_lnse"], dtype=np.float64)  # [128, 32]
        for b in range(BPC):
            lse_b = lnse[:, b * NTC : (b + 1) * NTC].sum()
            loglik = (
                np.log(a_tail[b, 0] + a_tail[b, 1])
                + eacc[112 + b] * ln2
                + T * ln2  # PRE = 0.5 correction
                - lse_b
            )
            losses.append(-loglik)
    return np.float32(np.mean(losses))
